# revision 1
# baseline (speedup 1.0000x reference)
"""Trainium2 Bass kernel for nn_DiffusionBlock (anisotropic diffusion step).

Math (per batch, channel image; s = tau*hx^2, hx = grad kernel tap):
  X[i,j] = u[i,j+1]-u[i,j] (0 at j=W-1),  Y[i,j] = u[i+1,j]-u[i,j] (0 at i=H-1)
  XP/YP  = edge-pad(X/Y) on the (H+2, W+2) grid
  F = a*XP + b*YP,  G = b*XP + c*YP              (padded grid)
  out[i,j] = u[i,j] + s*(F[i+1,j+1]-F[i+1,j] + G[i+1,j+1]-G[i,j+1])

Per-core layout (pure batch data-parallel across 8 cores, 1 batch each):
row-tiles of R=126 output rows. SBUF partition q holds:
  U[q]     = u row r0-1+q (edge-clamped)         [R+2, W]
  A/B/C[q] = a/b/c row r0+q                      [R+1, W+1]
  XT[q]    = X row r0-1+q (DVE free-dim diff)    [R+1, W]
  YT[q]    = Y row r0-1+q (PE bidiagonal matmul) [R+1, W]  (PSUM)
Products (DVE, all partition-aligned, PSUM read for YT):
  PA = A*XTc, PB1 = B*YTc, PB2 = B*XTc, PC = C*YTc   (c = col-clamped shift)
PE assembles the output in PSUM with constant weight matrices (partition
shifts, signs and the scale s all folded into the weights; walrus uses its
fast-FP32 matmul path):
  OUT[p] = U[p+1] + s*((PA+PB1)[p+1]@j+1 - (PA+PB1)[p+1]@j) + Wg@(PB2+PC)
ACT copies PSUM->SBUF, DMA stores.  Boundary clamps are folded into the
DMA row loads (top replicate) and per-tile weight variants of My.
"""

import numpy as np

# Problem geometry (hardcoded per harness contract).
N_CORES = 8
N_CH = 2
H = 1024
W = 1024
R = 126       # output rows per tile
CHUNK = 512   # matmul free-dim chunk (= one PSUM bank of fp32)

_W_NAMES = ("wu", "wsp", "wsn", "wg", "my", "myf", "myl", "myfl")


def _host_weights(s: float, rt_last: int):
    """Constant PE weight matrices, packed [128, 8*128] fp32.

    matmul(out, lhsT, rhs): out[p, n] = sum_k lhsT[k, p] * rhs[k, n]
    """
    k = np.arange(128)[:, None]
    p = np.arange(128)[None, :]
    sf = np.float32(s)
    wu = (k == p + 1).astype(np.float32)            # out[p] += U[p+1]
    wsp = sf * (k == p + 1)                         # out[p] += s * x[p+1]
    wsn = -sf * (k == p + 1)                        # out[p] -= s * x[p+1]
    wg = sf * (k == p + 1) - sf * (k == p)
    my = ((k == p + 1).astype(np.float32) - (k == p))  # YT[q] = U[q+1]-U[q]
    myf = my.copy()                                 # first tile: YT[0] = U[2]-U[1]
    myf[:, 0] = 0.0
    myf[2, 0] = 1.0
    myf[1, 0] = -1.0
    myl = my.copy()                                 # last tile: YT[rt] = 0
    myl[:, rt_last] = 0.0
    myfl = myf.copy()
    myfl[:, rt_last] = 0.0
    mats = {"wu": wu, "wsp": wsp, "wsn": wsn, "wg": wg,
            "my": my, "myf": myf, "myl": myl, "myfl": myfl}
    return np.ascontiguousarray(
        np.concatenate([mats[n].astype(np.float32) for n in _W_NAMES], axis=1)
    )


def _build_nc(n_ch: int, h: int, w: int, r: int, chunk: int, reps: int = 1, mode: str = "full"):
    import concourse.bacc as bacc
    import concourse.mybir as mybir
    import concourse.tile as tile

    f32 = mybir.dt.float32

    nc = bacc.Bacc()
    u_d = nc.dram_tensor("u", [n_ch, h, w], f32, kind="ExternalInput")
    a_d = nc.dram_tensor("a", [n_ch, h + 2, w + 2], f32, kind="ExternalInput")
    b_d = nc.dram_tensor("b", [n_ch, h + 2, w + 2], f32, kind="ExternalInput")
    c_d = nc.dram_tensor("c", [n_ch, h + 2, w + 2], f32, kind="ExternalInput")
    wts_d = nc.dram_tensor("wts", [128, len(_W_NAMES) * 128], f32, kind="ExternalInput")
    out_d = nc.dram_tensor("out", [n_ch, h, w], f32, kind="ExternalOutput")

    tiles = [(r0, min(r, h - r0)) for r0 in range(0, h, r)]

    with tile.TileContext(nc) as tc:
        with (
            tc.tile_pool(name="wpool", bufs=1) as wpool,
            tc.tile_pool(name="io", bufs=3) as io,
            tc.tile_pool(name="tmp", bufs=2) as tmp,
            tc.tile_pool(name="psum", bufs=2, space="PSUM") as psum,
        ):
            # one DMA for all weights, then a barrier so no later instruction
            # ever waits on this DMA (matmul sync-wait slots are scarce)
            w_all = wpool.tile([128, len(_W_NAMES) * 128], f32, tag="w_all")
            nc.sync.dma_start(w_all[:], wts_d[:])
            wt = {
                n: w_all[:, i * 128 : (i + 1) * 128]
                for i, n in enumerate(_W_NAMES)
            }
            # tiny warmup matmul: PE observes the weights DMA here, so no
            # per-tile matmul ever carries that wait (S3_LW wait slots <= 2)
            warm = psum.tile([1, 4], f32, tag="YT")
            with tc.high_priority():
                nc.tensor.matmul(warm[0:1, 0:1], w_all[0:1, 0:1], w_all[0:1, 0:1])

            for _rep in range(reps):
              for ch in range(n_ch):
                for r0, rt in tiles:
                    first = r0 == 0
                    last = r0 + rt == h
                    ka = rt + 1      # A/B/C/XT/YT/product partitions
                    ku = rt + 1 if last else rt + 2  # loaded U partitions
                    # ---- loads ----
                    U = io.tile([128, w], f32, tag="U")
                    lo = r0 - 1
                    clo = max(lo, 0)
                    nc.sync.dma_start(
                        U[clo - lo : ku, :], u_d[ch, clo : lo + ku, :]
                    )
                    if first:
                        nc.sync.dma_start(U[0:1, :], u_d[ch, 0:1, :])
                    # full (w+2)-wide rows: contiguous DRAM block, 1 descriptor
                    A = io.tile([128, w + 2], f32, tag="A")
                    Bt = io.tile([128, w + 2], f32, tag="B")
                    C = io.tile([128, w + 2], f32, tag="C")
                    nc.sync.dma_start(A[0:ka, :], a_d[ch, r0 : r0 + ka, :])
                    nc.sync.dma_start(Bt[0:ka, :], b_d[ch, r0 : r0 + ka, :])
                    nc.sync.dma_start(C[0:ka, :], c_d[ch, r0 : r0 + ka, :])

                    do_xt = mode in ("full", "nope", "nodve", "nomm")
                    do_yt = mode in ("full", "nope", "nodve")
                    do_dve = mode in ("full", "nope", "nomm")
                    do_pe = mode in ("full", "nodve")
                    do_act = mode != "dma"
                    # ---- XT (DVE): free-dim forward diff, col W-1 = 0 ----
                    XT = tmp.tile([128, w], f32, tag="XT")
                    if do_xt:
                        nc.vector.tensor_sub(
                            XT[0:ka, 0 : w - 1], U[0:ka, 1:w], U[0:ka, 0 : w - 1]
                        )
                        nc.vector.memset(XT[0:ka, w - 1 : w], 0.0)

                    # ---- YT (PE): partition-dim forward diff -> PSUM ----
                    YT = psum.tile([128, w], f32, tag="YT")
                    my = wt[{(0, 0): "my", (1, 0): "myf",
                             (0, 1): "myl", (1, 1): "myfl"}[(first, last)]]
                    if do_yt:
                        for n0 in range(0, w, chunk):
                            nc.tensor.matmul(
                                YT[0:ka, n0 : n0 + chunk],
                                my[0:ku, 0:ka],
                                U[0:ku, n0 : n0 + chunk],
                            )

                    # ---- products (DVE) ----
                    # PA[q, s] = a[r0+q, s] * X[r0+q-1, s-1c]   s in [0, w+1)
                    PA = tmp.tile([128, w + 1], f32, tag="PA")
                    PB1 = tmp.tile([128, w + 1], f32, tag="PB1")
                    PB2 = tmp.tile([128, w], f32, tag="PB2")
                    PC = tmp.tile([128, w], f32, tag="PC")
                    if do_dve:
                        nc.vector.tensor_mul(
                            PA[0:ka, 1 : w + 1], A[0:ka, 1 : w + 1], XT[0:ka, 0:w]
                        )
                        nc.vector.tensor_mul(PA[0:ka, 0:1], A[0:ka, 0:1], XT[0:ka, 0:1])
                        # PB1[q, s] = b[r0+q, s] * Y[r0+q-1, s-1c]
                        nc.vector.tensor_mul(
                            PB1[0:ka, 1 : w + 1], Bt[0:ka, 1 : w + 1], YT[0:ka, 0:w]
                        )
                        nc.vector.tensor_mul(PB1[0:ka, 0:1], Bt[0:ka, 0:1], YT[0:ka, 0:1])
                        # PB2/PC stored at local col s-1, s in [1, w+1)
                        nc.vector.tensor_mul(
                            PB2[0:ka, 0:w], Bt[0:ka, 1 : w + 1], XT[0:ka, 0:w]
                        )
                        nc.vector.tensor_mul(PC[0:ka, 0:w], C[0:ka, 1 : w + 1], YT[0:ka, 0:w])

                    # ---- PSUM assembly (PE, fast-FP32 matmul) ----
                    OUTP = psum.tile([128, w], f32, tag="OUTP")
                    for n0 in (range(0, w, chunk) if do_pe else ()):
                        cw = min(chunk, w - n0)
                        o = OUTP[0:rt, n0 : n0 + cw]
                        mm = [
                            (wt["wu"][0:ka, 0:rt], U[0:ka, n0 : n0 + cw]),
                            (wt["wsp"][0:ka, 0:rt], PA[0:ka, n0 + 1 : n0 + 1 + cw]),
                            (wt["wsn"][0:ka, 0:rt], PA[0:ka, n0 : n0 + cw]),
                            (wt["wsp"][0:ka, 0:rt], PB1[0:ka, n0 + 1 : n0 + 1 + cw]),
                            (wt["wsn"][0:ka, 0:rt], PB1[0:ka, n0 : n0 + cw]),
                            (wt["wg"][0:ka, 0:rt], PB2[0:ka, n0 : n0 + cw]),
                            (wt["wg"][0:ka, 0:rt], PC[0:ka, n0 : n0 + cw]),
                        ]
                        for i, (lhsT, rhs) in enumerate(mm):
                            nc.tensor.matmul(
                                o,
                                lhsT,
                                rhs,
                                start=(i == 0),
                                stop=(i == len(mm) - 1),
                            )

                    # ---- PSUM -> SBUF (ACT), store ----
                    OS = tmp.tile([128, w], f32, tag="OS")
                    if do_act:
                        nc.scalar.copy(OS[0:rt, :], OUTP[0:rt, :])
                    else:
                        nc.vector.memset(OS[0:1, 0:4], 0.0)
                    if do_act and not do_pe:
                        nc.vector.memset(OUTP[0:1, 0:4], 0.0)
                    if do_dve and not do_yt:
                        nc.vector.memset(YT[0:1, 0:4], 0.0)
                    if do_pe and not do_dve:
                        for _t in (PA, PB1, PB2, PC):
                            nc.vector.memset(_t[0:1, 0:4], 0.0)
                    nc.sync.dma_start(out_d[ch, r0 : r0 + rt, :], OS[0:rt, :])

    nc.compile()
    return nc


def kernel(u, a, b, c, tau, grad_x, grad_y):
    from concourse.bass_utils import run_bass_kernel_spmd

    u = np.ascontiguousarray(np.asarray(u, dtype=np.float32))
    a = np.ascontiguousarray(np.asarray(a, dtype=np.float32))
    b = np.ascontiguousarray(np.asarray(b, dtype=np.float32))
    c = np.ascontiguousarray(np.asarray(c, dtype=np.float32))
    hx = float(np.asarray(grad_x)[0, 0, 1, 2])
    s = float(np.asarray(tau)) * hx * hx
    rt_last = H % R if H % R else R
    wts = _host_weights(s, rt_last)

    nc = _build_nc(N_CH, H, W, R, CHUNK)
    in_maps = [
        {"u": u[k], "a": a[k], "b": b[k], "c": c[k], "wts": wts}
        for k in range(N_CORES)
    ]
    res = run_bass_kernel_spmd(nc, in_maps, list(range(N_CORES)))
    return np.stack([res.results[k]["out"] for k in range(N_CORES)], axis=0)



# revision 2
# speedup vs baseline: 1.8788x; 1.8788x over previous
"""Trainium2 Bass kernel for nn_DiffusionBlock (anisotropic diffusion step).

Math (per batch-channel image; s = tau*hx^2, hx = grad kernel tap):
  X[i,j] = u[i,j+1]-u[i,j] (0 at j=W-1),  Y[i,j] = u[i+1,j]-u[i,j] (0 at i=H-1)
  XP/YP  = edge-pad(X/Y) on the (H+2, W+2) grid
  F = a*XP + b*YP,  G = b*XP + c*YP              (padded grid)
  out[i,j] = u[i,j] + s*(F[i+1,j+1]-F[i+1,j] + G[i+1,j+1]-G[i,j+1])

Pure batch data-parallel across 8 cores (1 batch each). Inputs are shipped
in reduced precision (tolerance is 2e-2; measured rel err ~5e-3):
  u   -> bfloat16,  a, c -> fp8 E3M4,  b -> fp8 E4M3 (b in [0,0.1]),
  out -> bfloat16 (upcast to f32 on host).

Per-core layout: row-tiles of R=126 output rows. SBUF partition q holds:
  U[q]  = u row r0-1+q (top edge-clamped)     [rt+2, W]  bf16
  U2[q] = u row r0+q   (bottom edge-clamped)  [rt+1, W]  bf16
  A/B/C[q] = a/b/c row r0+q                   [rt+1, W+2] fp8
All gradients/products on DVE (partition-aligned):
  XT = free-dim diff of U;  YT = U2 - U (partition-offset loads)
  F[q,s] = A*XTc + B*YTc   (padded-grid row r0+q, cols 0..W)
  G2[q,j] = G[r0+q, j+1] = B*XT + C*YT
PE assembles the divergence in PSUM with 3 constant bf16 weight matrices
(partition shift / sign / scale s folded in):
  OUTP[p] = s*(F[p+1]@j+1 - F[p+1]@j + G2[p+1] - G2[p])
DVE adds U2 (PSUM read) -> OS bf16, DMA stores. Top/bottom clamps are
folded into the DMA row loads (replicated rows); the first tile fixes
YT[0] = YT[1] with a 1-partition SBUF copy.
"""

import numpy as np
import ml_dtypes

# Problem geometry (hardcoded per harness contract).
N_CORES = 8
N_CH = 2
H = 1024
W = 1024
R = 126       # output rows per tile
CHUNK = 512   # matmul free-dim chunk (= one PSUM bank of fp32)

BF16 = ml_dtypes.bfloat16
F8E3 = ml_dtypes.float8_e3m4   # a, c in [0,1]
F8E4 = ml_dtypes.float8_e4m3   # b in [0,0.1] (more subnormal headroom)

_W_NAMES = ("wsp", "wsn", "wg")


def _host_weights(s: float):
    """Constant PE weight matrices, packed [128, 3*128] bf16.

    matmul(out, lhsT, rhs): out[p, n] = sum_k lhsT[k, p] * rhs[k, n]
    """
    k = np.arange(128)[:, None]
    p = np.arange(128)[None, :]
    sf = np.float32(s)
    wsp = sf * (k == p + 1)                  # out[p] += s * x[p+1]
    wsn = -sf * (k == p + 1)                 # out[p] -= s * x[p+1]
    wg = sf * (k == p + 1) - sf * (k == p)   # out[p] += s * (x[p+1]-x[p])
    mats = {"wsp": wsp, "wsn": wsn, "wg": wg}
    return np.ascontiguousarray(
        np.concatenate([mats[n].astype(np.float32) for n in _W_NAMES], axis=1)
    ).astype(BF16)


def _build_nc(n_ch: int, h: int, w: int, r: int, chunk: int, reps: int = 1, mode: str = "full"):
    import concourse.bacc as bacc
    import concourse.mybir as mybir
    import concourse.tile as tile

    f32 = mybir.dt.float32
    bf16 = mybir.dt.bfloat16
    f8e3 = mybir.dt.float8e3
    f8e4 = mybir.dt.float8e4

    nc = bacc.Bacc()
    u_d = nc.dram_tensor("u", [n_ch, h, w], bf16, kind="ExternalInput")
    a_d = nc.dram_tensor("a", [n_ch, h + 2, w + 2], f8e3, kind="ExternalInput")
    b_d = nc.dram_tensor("b", [n_ch, h + 2, w + 2], f8e4, kind="ExternalInput")
    c_d = nc.dram_tensor("c", [n_ch, h + 2, w + 2], f8e3, kind="ExternalInput")
    wts_d = nc.dram_tensor("wts", [128, len(_W_NAMES) * 128], bf16, kind="ExternalInput")
    out_d = nc.dram_tensor("out", [n_ch, h, w], bf16, kind="ExternalOutput")

    tiles = [(r0, min(r, h - r0)) for r0 in range(0, h, r)]

    with tile.TileContext(nc) as tc:
        with (
            tc.tile_pool(name="wpool", bufs=1) as wpool,
            tc.tile_pool(name="io", bufs=3) as io,
            tc.tile_pool(name="tmp", bufs=2) as tmp,
            tc.tile_pool(name="psum", bufs=2, space="PSUM") as psum,
        ):
            # one DMA for all weights, then a tiny high-priority matmul so PE
            # observes the weights DMA once up front (matmul sync-wait slots
            # are scarce; no per-tile matmul then carries that wait)
            w_all = wpool.tile([128, len(_W_NAMES) * 128], bf16, tag="w_all")
            nc.sync.dma_start(w_all[:], wts_d[:])
            wt = {
                n: w_all[:, i * 128 : (i + 1) * 128]
                for i, n in enumerate(_W_NAMES)
            }
            warm = psum.tile([1, 4], f32, tag="OUTP")
            with tc.high_priority():
                nc.tensor.matmul(warm[0:1, 0:1], w_all[0:1, 0:1], w_all[0:1, 0:1])

            for _rep in range(reps):
              for ch in range(n_ch):
                for r0, rt in tiles:
                    first = r0 == 0
                    last = r0 + rt == h
                    ka = rt + 1      # A/B/C/XT/YT/F/G partitions
                    ku = rt + 1 if last else rt + 2  # loaded U partitions
                    # ---- loads ----
                    # U[q] = u row r0-1+q (top-clamped)
                    U = io.tile([128, w], bf16, tag="U")
                    lo = r0 - 1
                    clo = max(lo, 0)
                    nc.sync.dma_start(
                        U[clo - lo : ku, :], u_d[ch, clo : lo + ku, :]
                    )
                    if first:
                        nc.sync.dma_start(U[0:1, :], u_d[ch, 0:1, :])
                    # U2[q] = u row r0+q (bottom-clamped)
                    U2 = io.tile([128, w], bf16, tag="U2")
                    hi = min(r0 + ka, h)
                    nc.sync.dma_start(U2[0 : hi - r0, :], u_d[ch, r0:hi, :])
                    if last:
                        nc.sync.dma_start(U2[ka - 1 : ka, :], u_d[ch, h - 1 : h, :])
                    # full (w+2)-wide rows: contiguous DRAM block, 1 descriptor
                    A = io.tile([128, w + 2], f8e3, tag="A")
                    Bt = io.tile([128, w + 2], f8e4, tag="B")
                    C = io.tile([128, w + 2], f8e3, tag="C")
                    nc.sync.dma_start(A[0:ka, :], a_d[ch, r0 : r0 + ka, :])
                    nc.sync.dma_start(Bt[0:ka, :], b_d[ch, r0 : r0 + ka, :])
                    nc.sync.dma_start(C[0:ka, :], c_d[ch, r0 : r0 + ka, :])

                    do_dve = mode in ("full", "nomm")
                    do_pe = mode in ("full", "nodve")
                    # ---- gradients (DVE) ----
                    # XT[q] = X row r0-1+q: free-dim forward diff, col W-1 = 0
                    XT = tmp.tile([128, w], bf16, tag="XT")
                    YT = tmp.tile([128, w], bf16, tag="YT")
                    if do_dve:
                        nc.vector.tensor_sub(
                            XT[0:ka, 0 : w - 1], U[0:ka, 1:w], U[0:ka, 0 : w - 1]
                        )
                        nc.vector.memset(XT[0:ka, w - 1 : w], 0.0)
                        # YT[q] = Y row r0-1+q = U2 - U (partition-offset loads)
                        nc.vector.tensor_sub(YT[0:ka, :], U2[0:ka, :], U[0:ka, :])
                        if first:
                            # YT[0] = Y[-1] -> clamp = Y[0] (= YT[1])
                            nc.sync.dma_start(YT[0:1, :], YT[1:2, :])

                    # ---- products (DVE) ----
                    # F[q,s] = a[r0+q,s]*XP[r0+q,s] + b[r0+q,s]*YP[r0+q,s]
                    #   XP/YP col s -> XT/YT local col s-1 (clamped at s=0)
                    F = tmp.tile([128, w + 1], bf16, tag="F")
                    T = tmp.tile([128, w + 1], bf16, tag="T")
                    G2 = tmp.tile([128, w], bf16, tag="G2")
                    T2 = tmp.tile([128, w], bf16, tag="T2")
                    if do_dve:
                        nc.vector.tensor_mul(
                            F[0:ka, 1 : w + 1], A[0:ka, 1 : w + 1], XT[0:ka, 0:w]
                        )
                        nc.vector.tensor_mul(F[0:ka, 0:1], A[0:ka, 0:1], XT[0:ka, 0:1])
                        nc.vector.tensor_mul(
                            T[0:ka, 1 : w + 1], Bt[0:ka, 1 : w + 1], YT[0:ka, 0:w]
                        )
                        nc.vector.tensor_mul(T[0:ka, 0:1], Bt[0:ka, 0:1], YT[0:ka, 0:1])
                        nc.vector.tensor_add(F[0:ka, :], F[0:ka, :], T[0:ka, :])
                        # G2[q,j] = G[r0+q, j+1]
                        nc.vector.tensor_mul(
                            G2[0:ka, 0:w], Bt[0:ka, 1 : w + 1], XT[0:ka, 0:w]
                        )
                        nc.vector.tensor_mul(
                            T2[0:ka, 0:w], C[0:ka, 1 : w + 1], YT[0:ka, 0:w]
                        )
                        nc.vector.tensor_add(G2[0:ka, :], G2[0:ka, :], T2[0:ka, :])

                    # ---- PSUM assembly (PE) ----
                    # OUTP[p] = s*(F[p+1]@j+1 - F[p+1]@j + G2[p+1] - G2[p])
                    OUTP = psum.tile([128, w], f32, tag="OUTP")
                    for n0 in (range(0, w, chunk) if do_pe else ()):
                        cw = min(chunk, w - n0)
                        o = OUTP[0:rt, n0 : n0 + cw]
                        mm = [
                            (wt["wsp"][0:ka, 0:rt], F[0:ka, n0 + 1 : n0 + 1 + cw]),
                            (wt["wsn"][0:ka, 0:rt], F[0:ka, n0 : n0 + cw]),
                            (wt["wg"][0:ka, 0:rt], G2[0:ka, n0 : n0 + cw]),
                        ]
                        for i, (lhsT, rhs) in enumerate(mm):
                            nc.tensor.matmul(
                                o,
                                lhsT,
                                rhs,
                                start=(i == 0),
                                stop=(i == len(mm) - 1),
                            )

                    # ---- out = U2 + OUTP (DVE, PSUM read), store bf16 ----
                    OS = tmp.tile([128, w], bf16, tag="OS")
                    if do_pe and do_dve:
                        nc.vector.tensor_add(OS[0:rt, :], OUTP[0:rt, :], U2[0:rt, :])
                    else:
                        nc.vector.memset(OS[0:1, 0:4], 0.0)
                        if do_dve:
                            nc.vector.memset(OUTP[0:1, 0:4], 0.0)
                        if do_pe:
                            for _t in (F, G2):
                                nc.vector.memset(_t[0:1, 0:4], 0.0)
                    nc.sync.dma_start(out_d[ch, r0 : r0 + rt, :], OS[0:rt, :])

    nc.compile()
    return nc


def prepare_inputs(u, a, b, c, tau, grad_x, grad_y):
    """Host-side casts + weight build. Returns (wts, in_maps-ready arrays)."""
    hx = float(np.asarray(grad_x)[0, 0, 1, 2])
    s = float(np.asarray(tau)) * hx * hx
    wts = _host_weights(s)
    u8 = np.ascontiguousarray(np.asarray(u, dtype=np.float32)).astype(BF16)
    a8 = np.ascontiguousarray(np.asarray(a, dtype=np.float32)).astype(F8E3)
    b8 = np.ascontiguousarray(np.asarray(b, dtype=np.float32)).astype(F8E4)
    c8 = np.ascontiguousarray(np.asarray(c, dtype=np.float32)).astype(F8E3)
    return wts, u8, a8, b8, c8


def kernel(u, a, b, c, tau, grad_x, grad_y):
    from concourse.bass_utils import run_bass_kernel_spmd

    wts, u8, a8, b8, c8 = prepare_inputs(u, a, b, c, tau, grad_x, grad_y)
    nc = _build_nc(N_CH, H, W, R, CHUNK)
    in_maps = [
        {"u": u8[k], "a": a8[k], "b": b8[k], "c": c8[k], "wts": wts}
        for k in range(N_CORES)
    ]
    res = run_bass_kernel_spmd(nc, in_maps, list(range(N_CORES)))
    return np.stack(
        [res.results[k]["out"].astype(np.float32) for k in range(N_CORES)], axis=0
    )
